# revision 29
# baseline (speedup 1.0000x reference)
"""Trainium2 Bass kernel for nn_CPCircuitLayer (embedding_lookup).

Math: A_b = X_b @ W_seq^T [S,R]; Bm_b = X_b^T @ W_hid^T [H,R]
      out[b, n] = dot(A_b[idx_s[n]], Bm_b[idx_h[n]]),  out -> [B, S, H]

Key reformulation: out[b, n] = G_b[idx_s[n], idx_h[n]] where
G_b = A_b @ Bm_b^T is a [S, H] = [1024, 1024] f32 matrix that fits in
SBUF (tiny matmul: S*H*R = 34M MACs). The problem becomes a scalar
gather of N entries from G. Since idx pairs are batch-independent, both
batches' tables are interleaved in SBUF ([128, 8192, 2] f32, partition
p = s'%128, e = (s'//128)*1024 + h) and a single d=2 ap_gather index
fetches BOTH batches' output values: 2 outputs per index.

ap_gather costs ~27ns per index per 16-partition group (measured),
independent of d/num_elems, so index count is everything: 131072
idx/group (baseline) -> 16640 here (7.9x).

Load balancing: the host permutes X's rows (and W_hid's columns to
match) per core so the 1024 s-rows pack into 128 partitions with
near-equal gather-bucket sizes -> pad L=1040 (~1.5% waste).

Sharding: core c handles n in [c*N/8, (c+1)*N/8) for both batches.
Host buckets each core's 131072 outputs by partition p, pads each
bucket to L, streams group g's 16 buckets lane-by-lane; round r
gathers lane r's L indices for all groups and a static block-indicator
matmul (lhsT = ind[:, 8r:8r+8]) extracts lane r of each group ->
psum [8, 512]-chunks -> out. Host inverse-permutes the bucketed
outputs (pure data movement).

X^T (needed for the A factor) is produced on the PE via transpose
matmuls for batch 0 while batch 1 is DMA-transposed concurrently.
"""

import numpy as np
import ml_dtypes
from contextlib import ExitStack

import concourse.bass as bass
import concourse.mybir as mybir
import concourse.tile as tile
from concourse import bacc

B, S, H, R = 2, 1024, 1024, 32
N = S * H
NCORES = 8
J0 = N // NCORES          # 131072 n-indices per core (serves both batches)
L = 976                   # padded per-partition bucket length; buckets
                          # hold DEDUPED (s', h) cells (~962 mean)
NE = 8 * 1024             # d=2 table blocks per partition
OUTW = 16 * L * 2         # 31232 output cols per core: [8, OUTW]
IDXC = 16 * L // 16 + 1   # 977 idx cols; one pad col before lane 15 so
                          # its solo round starts 4B-aligned (L/16 is odd)
# gather rounds as lane tuples; the last two lanes go solo so their
# extraction/writeback tail hides under the preceding gather
ROUNDS = [(2 * i, 2 * i + 1) for i in range(7)] + [(14,), (15,)]


def _col0(lane: int) -> int:
    return lane * (L // 16) + (1 if lane == 15 else 0)

F32 = mybir.dt.float32
BF16 = mybir.dt.bfloat16
I16 = mybir.dt.int16


def _build(reps: int = 1):
    nc = bacc.Bacc()
    x0 = nc.declare_dram_parameter("x0", [S, H], BF16, False)
    x1 = nc.declare_dram_parameter("x1", [S, H], BF16, False)
    xt0 = nc.declare_dram_parameter("xt0", [H, S], BF16, False)
    xt1 = nc.declare_dram_parameter("xt1", [H, S], BF16, False)
    # already row-scattered on host: [p, k, r] = W^T[p + 128k, r]
    wseq_t = nc.declare_dram_parameter("wseq_t", [128, 8 * R], BF16, False)
    whid_t = nc.declare_dram_parameter("whid_t", [128, 8 * R], BF16, False)
    idx = nc.declare_dram_parameter("idx", [128, IDXC], I16, False)
    ind_in = nc.declare_dram_parameter("ind", [128, 128], BF16, False)
    out = nc.declare_dram_parameter("out", [8, OUTW], F32, True)
    xs = (x0, x1)
    xts = (xt0, xt1)

    with tile.TileContext(nc) as tc, ExitStack() as ctx:
        base = ctx.enter_context(tc.tile_pool(name="base", bufs=1))
        fps = ctx.enter_context(tc.tile_pool(name="fps", bufs=1, space="PSUM"))
        gps = ctx.enter_context(tc.tile_pool(name="gps", bufs=2, space="PSUM"))
        rps = ctx.enter_context(tc.tile_pool(name="rps", bufs=2, space="PSUM"))
        tabp = ctx.enter_context(tc.tile_pool(name="tabp", bufs=1))
        facp = ctx.enter_context(tc.tile_pool(name="facp", bufs=1))
        gap = ctx.enter_context(tc.tile_pool(name="gap", bufs=2))
        otp = ctx.enter_context(tc.tile_pool(name="otp", bufs=2))

        # --- static loads -----------------------------------------------
        ws_sb = base.tile([128, 8, R], BF16)     # W_seq^T rows, h-major
        wh_sb = base.tile([128, 8, R], BF16)     # W_hid^T rows, s-major
        isb = base.tile([128, IDXC], I16)
        ind_sb = base.tile([128, 128], BF16)     # ind[p, 8*l+g]=1 iff p==16g+l
        x_sb = base.tile([128, 2, 8, H], BF16)   # [p, b, k, h]; s' = p + 128k
        xt_sb = base.tile([128, 2, 8, S], BF16)  # [p, b, c, s]; h = p + 128c

        # small tensors first (w/ind on SP, idx on ACT), then the bulk:
        # x chunks on the SP queue, host-pretransposed x^T chunks on the
        # ACT queue — the streams load concurrently, PE starts on chunk 0
        nc.sync.dma_start(out=ws_sb[:], in_=wseq_t[:])
        nc.sync.dma_start(out=wh_sb[:], in_=whid_t[:])
        nc.sync.dma_start(out=ind_sb[:], in_=ind_in[:])
        nc.scalar.dma_start(out=isb[:], in_=idx[:])
        for b in range(B):
            for k in range(8):
                nc.sync.dma_start(
                    out=x_sb[:, b, k, :],
                    in_=bass.AP(tensor=xs[b][:].tensor, offset=128 * k * H,
                                ap=[[H, 128], [1, H]]),
                )
                nc.scalar.dma_start(
                    out=xt_sb[:, b, k, :],
                    in_=bass.AP(tensor=xts[b][:].tensor, offset=128 * k * S,
                                ap=[[S, 128], [1, S]]),
                )

        for _ in range(reps):
            _body(nc, fps, gps, rps, tabp, facp, gap, otp,
                  ws_sb, wh_sb, isb, ind_sb, x_sb, xt_sb, out)
    nc.compile()
    return nc


def _body(nc, fps, gps, rps, tabp, facp, gap, otp,
          ws_sb, wh_sb, isb, ind_sb, x_sb, xt_sb, out):
    tab = tabp.tile([128, 2 * NE], BF16, tag="tab")  # tab[p, 2e+b]
    a_bf = facp.tile([32, 2, S], BF16, tag="a_bf")   # A_b^T[r, s']
    b_bf = facp.tile([32, 2, H], BF16, tag="b_bf")   # Bm_b^T[r, h]

    # --- per batch: factors F^T [32, 1024], then G_b ------------------
    # G block k covers s' in [128k, 128k+128): out partition i = s'-128k,
    # table col e = 1024k + h, written at tab[:, 2e + b] (stride 2).
    eng = 0
    for b in range(B):
        for (dst, lhs_w, rhs_x) in ((b_bf, wh_sb, x_sb), (a_bf, ws_sb, xt_sb)):
            pt = fps.tile([R, 1024], F32, tag="pt")
            for k in range(8):
                for nh in range(2):
                    nc.tensor.matmul(
                        out=pt[:, nh * 512:(nh + 1) * 512],
                        lhsT=lhs_w[:, k, :],
                        rhs=rhs_x[:, b, k, nh * 512:(nh + 1) * 512],
                        start=(k == 0), stop=(k == 7),
                    )
            if dst is b_bf:
                nc.scalar.copy(out=dst[:, b, :], in_=pt[:])
            else:
                nc.vector.tensor_copy(out=dst[:, b, :], in_=pt[:])
        for k in range(8):
            for nh in range(2):
                gp = gps.tile([128, 512], F32, tag="gp")
                nc.tensor.matmul(
                    out=gp[:],
                    lhsT=a_bf[:, b, 128 * k:128 * (k + 1)],
                    rhs=b_bf[:, b, 512 * nh:512 * (nh + 1)],
                    start=True, stop=True,
                )
                dst = bass.AP(
                    tensor=tab[:].tensor,
                    offset=tab[:].offset + 2 * (1024 * k + 512 * nh) + b,
                    ap=[list(tab[:].ap[0]), [2, 512]],
                )
                if eng % 2 == 0:
                    nc.vector.tensor_copy(out=dst, in_=gp[:])
                else:
                    nc.scalar.copy(out=dst, in_=gp[:])
                eng += 1

    # --- gather + extract ----------------------------------------------
    # a round gathers its lanes' buckets back to back (2L idxs per lane
    # pair); extraction picks lane lanes[i] for cols [2L*i, 2L*(i+1)).
    tab_flat = bass.AP(tensor=tab[:].tensor, offset=tab[:].offset,
                       ap=[list(tab[:].ap[0]), [1, 2 * NE], [1, 1]])
    for lanes in ROUNDS:
        nl = len(lanes)
        gw = nl * 2 * L
        ga = gap.tile([128, 4 * L], BF16, tag="ga")
        ga_ap = bass.AP(tensor=ga[:].tensor, offset=ga[:].offset,
                        ap=[list(ga[:].ap[0]), [1, gw], [1, 1]])
        c0 = _col0(lanes[0])
        nc.gpsimd.ap_gather(
            out_ap=ga_ap, in_ap=tab_flat,
            idxs_ap=isb[:, c0:c0 + nl * (L // 16)],
            channels=128, num_elems=NE, d=2, num_idxs=nl * L,
        )
        ot = otp.tile([8, 4 * L], F32, tag="ot")
        for i, lane in enumerate(lanes):
            for t in range(i * 2 * L, (i + 1) * 2 * L, 512):
                w = min(512, (i + 1) * 2 * L - t)
                rp = rps.tile([8, 512], F32, tag="rp")
                nc.tensor.matmul(
                    out=rp[:, :w],
                    lhsT=ind_sb[:, 8 * lane:8 * (lane + 1)],
                    rhs=ga[:, t:t + w],
                    start=True, stop=True,
                )
                nc.scalar.copy(out=ot[:, t:t + w], in_=rp[:, :w])
        nc.sync.dma_start(
            out=bass.AP(tensor=out[:].tensor, offset=lanes[0] * 2 * L,
                        ap=[[OUTW, 8], [1, gw]]),
            in_=ot[:, :gw],
        )


_nc_cache_by_reps = {}


def _get_nc(reps: int = 1):
    nc = _nc_cache_by_reps.get(reps)
    if nc is None:
        nc = _nc_cache_by_reps[reps] = _build(reps)
    return nc


class _Runner:
    """Trace/compile the SPMD executable once; reuse across calls."""

    def __init__(self, nc):
        import jax
        from jax.experimental.shard_map import shard_map
        from jax.sharding import Mesh, PartitionSpec
        import concourse.bass2jax as b2j

        b2j.install_neuronx_cc_hook()
        self.nc = nc
        part_name = (nc.partition_id_tensor.name
                     if nc.partition_id_tensor else None)
        in_names, out_names, out_avals = [], [], []
        zero_outs = []
        for alloc in nc.m.functions[0].allocations:
            if not isinstance(alloc, mybir.MemoryLocationSet):
                continue
            name = alloc.memorylocations[0].name
            if alloc.kind == "ExternalInput":
                if name != part_name:
                    in_names.append(name)
            elif alloc.kind == "ExternalOutput":
                out_names.append(name)
                shape = tuple(alloc.tensor_shape)
                dtype = mybir.dt.np(alloc.dtype)
                out_avals.append(jax.core.ShapedArray(shape, dtype))
                zero_outs.append(np.zeros(shape, dtype))
        self.in_names = list(in_names)
        self.out_names = out_names
        self.zero_outs = zero_outs
        n_params = len(in_names)
        n_outs = len(out_names)
        all_in_names = in_names + out_names
        if part_name is not None:
            all_in_names = all_in_names + [part_name]
        donate = tuple(range(n_params, n_params + n_outs))

        def _body_fn(*args):
            operands = list(args)
            if part_name is not None:
                operands.append(b2j.partition_id_tensor())
            outs = b2j._bass_exec_p.bind(
                *operands,
                out_avals=tuple(out_avals),
                in_names=tuple(all_in_names),
                out_names=tuple(out_names),
                lowering_input_output_aliases=(),
                sim_require_finite=True,
                sim_require_nnan=True,
                nc=nc,
            )
            return tuple(outs)

        devices = jax.devices()[:NCORES]
        mesh = Mesh(np.asarray(devices), ("core",))
        self.fn = jax.jit(
            shard_map(
                _body_fn, mesh=mesh,
                in_specs=(PartitionSpec("core"),) * (n_params + n_outs),
                out_specs=(PartitionSpec("core"),) * n_outs,
                check_rep=False,
            ),
            donate_argnums=donate,
            keep_unused=True,
        )

    def __call__(self, in_maps):
        concat_in = [
            np.concatenate([np.asarray(m[name]) for m in in_maps], axis=0)
            for name in self.in_names
        ]
        concat_zeros = [
            np.zeros((NCORES * z.shape[0], *z.shape[1:]), z.dtype)
            for z in self.zero_outs
        ]
        out_arrs = self.fn(*concat_in, *concat_zeros)
        return [
            {
                name: np.asarray(out_arrs[i]).reshape(NCORES, -1)[c]
                for i, name in enumerate(self.out_names)
            }
            for c in range(NCORES)
        ]


_runner_cache = {}


def _get_runner(reps: int = 1):
    r = _runner_cache.get(reps)
    if r is None:
        r = _runner_cache[reps] = _Runner(_get_nc(reps))
    return r


def _make_ind() -> np.ndarray:
    ind = np.zeros((128, 128), ml_dtypes.bfloat16)
    for g in range(8):
        for l in range(16):
            ind[16 * g + l, 8 * l + g] = 1.0
    return ind


def _balance_rows(rc: np.ndarray) -> np.ndarray:
    """Assign the 1024 s-rows to 128 partitions (8 rows each) balancing
    total counts rc. Returns sigma: sigma[s'] = original row at permuted
    position s' (partition p = s'%128, slot j = s'//128)."""
    order = np.argsort(-rc, kind="stable")
    bins = np.zeros(128, np.int64)
    slots = np.zeros(128, np.int64)
    sigma = np.empty(S, np.int64)
    for row in order:
        cand = np.flatnonzero(slots < 8)
        p = cand[np.argmin(bins[cand])]
        sigma[p + 128 * slots[p]] = row
        bins[p] += rc[row]
        slots[p] += 1
    return sigma


def _prep_core(s: np.ndarray, h: np.ndarray):
    """Dedup, balance + bucket one core's J0 (s, h) pairs.

    Returns (sigma, idx_dev [128, IDXC] int16, meta for unpacking).
    """
    ukey, inv = np.unique(s * 1024 + h, return_inverse=True)
    us, uh = ukey >> 10, ukey & 1023
    sigma = _balance_rows(np.bincount(us, minlength=S))
    invpos = np.empty(S, np.int64)
    invpos[sigma] = np.arange(S)
    usp = invpos[us]                     # permuted row position s'
    # unique cells sorted by (partition, e): bucket order
    up = usp & 127
    ue = ((usp >> 7) << 10) | uh         # (s'//128)*1024 + h < 8192
    uord = np.argsort(up * 8192 + ue, kind="stable")
    up_s, ue_s = up[uord], ue[uord]
    cnt = np.bincount(up_s, minlength=128)
    starts = np.concatenate(([0], np.cumsum(cnt)[:-1]))
    ofs = np.arange(len(ukey)) - starts[up_s]
    keep = ofs < L
    arr = np.zeros((128, L), np.int16)
    arr[up_s[keep], ofs[keep]] = ue_s[keep].astype(np.int16)
    idx_dev = np.zeros((128, IDXC), np.int16)
    for g in range(8):
        for lane in range(16):
            c0 = _col0(lane)
            idx_dev[16 * g:16 * (g + 1), c0:c0 + L // 16] = \
                arr[16 * g + lane].reshape(L // 16, 16).T
    # slot_of_n: original n -> its unique slot in uord order
    slotmap = np.empty(len(ukey), np.int64)
    slotmap[uord] = np.arange(len(ukey))
    slot_of_n = slotmap[inv]
    return sigma, np.ascontiguousarray(idx_dev), \
        (slot_of_n, up_s, ofs, keep)


def prepare_in_maps(hidden_states, W_seq, W_hid, all_indices):
    hidden_states = np.asarray(hidden_states)
    all_indices = np.asarray(all_indices)
    x_bf = [hidden_states[b].astype(ml_dtypes.bfloat16) for b in range(B)]
    # W^T row-scattered to the device layout [p, k, r] = W^T[p + 128k, r]
    ws_t = np.ascontiguousarray(
        np.asarray(W_seq).T.astype(ml_dtypes.bfloat16)
        .reshape(8, 128, R).transpose(1, 0, 2).reshape(128, 8 * R))
    wh_t_full = np.asarray(W_hid).T.astype(ml_dtypes.bfloat16)  # [S, R]
    ind = _make_ind()
    in_maps, metas = [], []
    for c in range(NCORES):
        sl = slice(c * J0, (c + 1) * J0)
        s = all_indices[sl, 0].astype(np.int64)
        h = all_indices[sl, 1].astype(np.int64)
        sigma, idx_dev, meta = _prep_core(s, h)
        metas.append(meta)
        xp = [np.ascontiguousarray(x_bf[b][sigma]) for b in range(B)]
        in_maps.append({
            "x0": xp[0], "x1": xp[1],
            "xt0": np.ascontiguousarray(xp[0].T),
            "xt1": np.ascontiguousarray(xp[1].T),
            "wseq_t": ws_t,
            "whid_t": np.ascontiguousarray(
                wh_t_full[sigma]
                .reshape(8, 128, R).transpose(1, 0, 2).reshape(128, 8 * R)),
            "idx": idx_dev, "ind": ind,
        })
    return in_maps, metas


def _assemble(results, metas, hidden_states, all_indices):
    out_full = np.empty((B, N), dtype=np.float32)
    spill = []
    for c in range(NCORES):
        slot_of_n, up_s, ofs, keep = metas[c]
        resh = np.asarray(results[c]["out"]).reshape(8, 16, L, 2)
        # value of every unique slot (junk where dropped), then scatter
        # to all n via the dedup inverse map
        uvals = np.zeros((len(up_s), 2), np.float32)
        uvals[keep] = resh[up_s[keep] >> 4, up_s[keep] & 15, ofs[keep], :]
        vals = uvals[slot_of_n]                       # [J0, 2]
        out_full[0, c * J0:(c + 1) * J0] = vals[:, 0]
        out_full[1, c * J0:(c + 1) * J0] = vals[:, 1]
        if not keep.all():
            bad = ~keep[slot_of_n]
            spill.append(c * J0 + np.flatnonzero(bad))
    if spill:
        # exact host fallback for (astronomically rare) bucket overflow
        ns = np.concatenate(spill)
        s = all_indices[ns, 0].astype(np.int64)
        h = all_indices[ns, 1].astype(np.int64)
        for b in range(B):
            A = hidden_states[b].astype(np.float32) @ np.asarray(
                _W_CACHE["W_seq"]).T.astype(np.float32)
            Bm = hidden_states[b].astype(np.float32).T @ np.asarray(
                _W_CACHE["W_hid"]).T.astype(np.float32)
            out_full[b, ns] = np.sum(A[s] * Bm[h], axis=-1)
    return out_full.reshape(B, S, H)


_W_CACHE = {}


def kernel(hidden_states, W_seq, W_hid, all_indices):
    hidden_states = np.asarray(hidden_states)
    W_seq = np.asarray(W_seq)
    W_hid = np.asarray(W_hid)
    all_indices = np.asarray(all_indices)
    _W_CACHE["W_seq"] = W_seq
    _W_CACHE["W_hid"] = W_hid

    runner = _get_runner()
    in_maps, metas = prepare_in_maps(hidden_states, W_seq, W_hid, all_indices)
    results = runner(in_maps)
    return _assemble(results, metas, hidden_states, all_indices)
